# revision 1
# baseline (speedup 1.0000x reference)
"""Fused pre-norm attention layer (B=4, L=2048, D=1024, H=16, E=64) on 8 trn2 cores.

Sharding: core c handles batch b=c//2, query-row half h=c%2 (1024 rows).
K/V work is duplicated between the two cores of a batch (no collectives).

Device pipeline (per core), all layouts chosen so every per-row affine is a
per-partition op:
  - natural-layout uploads feed bn_stats for LayerNorm stats
  - transposed uploads feed projection matmuls (lhsT = x^T tiles)
  - LN is folded into the projections: norm_w folded into W, mean/bias folded
    via 2 augmented contraction rows, rstd applied as per-partition scale on
    the PSUM->SBUF copy-out (output in natural [row, feat] layout)
  - Q/K transposed on PE (identity matmul) to [feat, row] bf16 for attention
  - scores^T psum -> ACT exp (scale=1/8) -> P^T bf16
  - AV as O[l,65] with a ones-column in V_aug giving the softmax denominator
    as psum column 64; normalize via per-partition reciprocal+scale
  - out1 accumulates residual Q + sum_h A_h V_h in natural layout
  - final LN folded into Wo the same way; residual added on copy-out
  - output written as F^T [D, L-half]; host transposes back
"""

import numpy as np

import concourse.bass as bass
import concourse.mybir as mybir
import concourse.tile as tile
from concourse import bacc
from concourse.bass_utils import run_bass_kernel_spmd
from concourse.masks import make_identity

F32 = mybir.dt.float32
F32R = mybir.dt.float32r
BF16 = mybir.dt.bfloat16
AF = mybir.ActivationFunctionType

B, L, D, H, E = 4, 2048, 1024, 16, 64
LH = L // 2          # query rows per core
S = L                # key/value rows
HE = H * E           # 1024
EPS = 1e-5
NKT = D // 128       # 8 contraction k-tiles
NLT = LH // 128      # 8 query row tiles
NST = S // 128       # 16 key row tiles
NDT = D // 128       # 8 output-feature tiles
SCALE = 1.0 / np.sqrt(E)


def _prep_weight(nc, wk, sm, psp, res, Wd, wvec_c, bvec_c, bias_r, wp, tag):
    """Stream W, fold norm weight, build aug rhs rows [S'; c]. Returns aug tile."""
    n = Wd.shape[1]
    ps_c = psp.tile([1, n], F32, tag="proj0", name="wrow")
    ps_s = psp.tile([1, n], F32, tag="proj1", name="wrow2")
    onescol = sm.tile([128, 1], wp.dtype, tag="onescol", name="onescol")
    nc.vector.memset(onescol[:].bitcast(F32) if wp.dtype == F32R else onescol[:], 1.0)
    for k in range(NKT):
        wtile = wk.tile([128, n], F32R, tag="wtile", name="wtile")
        nc.sync.dma_start(wtile[:], Wd[k * 128 : (k + 1) * 128, :])
        for c in range(2):
            nc.tensor.matmul(
                ps_c[:, c * 512 : (c + 1) * 512],
                bvec_c[:, k : k + 1],
                wtile[:, c * 512 : (c + 1) * 512],
                start=(k == 0),
                stop=(k == NKT - 1),
            )
        nc.vector.tensor_scalar_mul(wp[:, k, :], wtile[:], wvec_c[:, k : k + 1])
        for c in range(2):
            nc.tensor.matmul(
                ps_s[:, c * 512 : (c + 1) * 512],
                onescol[:],
                wp[:, k, c * 512 : (c + 1) * 512],
                start=(k == 0),
                stop=(k == NKT - 1),
            )
    ar = res.tile([2, n], BF16, tag="augr", name=f"augr_{tag}")
    nc.vector.tensor_copy(ar[0:1, :], ps_s[:])
    brow = sm.tile([1, n], F32, tag="brow", name="brow")
    nc.sync.dma_start(brow[:], bias_r.ap())
    crow = sm.tile([1, n], BF16, tag="crow", name="crow")
    nc.vector.tensor_add(crow[:], ps_c[:], brow[:])
    nc.sync.dma_start(ar[1:2, :], crow[:])
    return ar


def build_nc(skip=()):
    nc = bacc.Bacc("TRN2", target_bir_lowering=False, debug=False, num_devices=8)

    qT = nc.dram_tensor("qT", [D, LH], F32R, kind="ExternalInput")
    kT = nc.dram_tensor("kT", [D, S], F32R, kind="ExternalInput")
    vT = nc.dram_tensor("vT", [D, S], F32R, kind="ExternalInput")
    qn_ = nc.dram_tensor("qn_", [LH, D], F32, kind="ExternalInput")
    kn_ = nc.dram_tensor("kn_", [S, D], F32, kind="ExternalInput")
    vn_ = nc.dram_tensor("vn_", [S, D], F32, kind="ExternalInput")
    Wq = nc.dram_tensor("Wq", [D, HE], F32R, kind="ExternalInput")
    Wk = nc.dram_tensor("Wk", [D, HE], F32R, kind="ExternalInput")
    Wv = nc.dram_tensor("Wv", [D, HE], F32R, kind="ExternalInput")
    Wo = nc.dram_tensor("Wo", [HE, D], F32R, kind="ExternalInput")
    nw_c = nc.dram_tensor("nw_c", [D, 1], F32, kind="ExternalInput")
    nb_c = nc.dram_tensor("nb_c", [D, 1], F32, kind="ExternalInput")
    n1w_c = nc.dram_tensor("n1w_c", [HE, 1], F32, kind="ExternalInput")
    n1b_c = nc.dram_tensor("n1b_c", [HE, 1], F32, kind="ExternalInput")
    bq_r = nc.dram_tensor("bq_r", [1, HE], F32, kind="ExternalInput")
    bk_r = nc.dram_tensor("bk_r", [1, HE], F32, kind="ExternalInput")
    bv_r = nc.dram_tensor("bv_r", [1, HE], F32, kind="ExternalInput")
    bo_r = nc.dram_tensor("bo_r", [1, D], F32, kind="ExternalInput")
    FT = nc.dram_tensor("FT", [D, LH], F32, kind="ExternalOutput")
    out1d = nc.dram_tensor("out1d", [LH, HE], F32R)

    with tile.TileContext(nc) as tc, nc.allow_low_precision(
        reason="float32r is rounded fp32; intentional"
    ):
        with (
            tc.tile_pool(name="res", bufs=1) as res,
            tc.tile_pool(name="wk", bufs=2) as wk,
            tc.tile_pool(name="xs", bufs=2) as xs,
            tc.tile_pool(name="nat", bufs=1) as natp,
            tc.tile_pool(name="sm", bufs=1) as sm,
            tc.tile_pool(name="ps", bufs=1, space="PSUM") as ps,
            tc.tile_pool(name="ps1", bufs=1, space="PSUM") as ps1,
        ):
            identf = res.tile([128, 128], F32, tag="identf", name="identf")
            make_identity(nc, identf[:])
            ident = res.tile([128, 128], F32R, tag="ident", name="ident")
            nc.vector.tensor_copy(ident[:], identf[:])
            identb = res.tile([128, 128], BF16, tag="identb", name="identb")
            nc.vector.tensor_copy(identb[:], identf[:])

            out1 = [
                res.tile([128, HE], F32R, tag=f"out1_{t}", name=f"out1_{t}")
                for t in range(NLT)
            ]

            nwc = res.tile([128, NKT], F32, tag="nwc", name="nwc")
            nc.sync.dma_start(nwc[:], nw_c.ap().rearrange("(k p) 1 -> p k", p=128))
            nbc = res.tile([128, NKT], F32R, tag="nbc", name="nbc")
            nc.gpsimd.dma_start(nbc[:], nb_c.ap().rearrange("(k p) 1 -> p k", p=128))

            # ---------- LN stats for q, k, v ----------
            stats = {}
            stp_cm = tc.tile_pool(name="stp", bufs=6)
            stp = stp_cm.__enter__()
            for name, xd, T in (("q", qn_, NLT), ("k", kn_, NST), ("v", vn_, NST)):
                mv = res.tile([128, T, 2], F32, tag=f"mv_{name}", name=f"mv_{name}")
                for t in range(T):
                    xt = stp.tile([128, D], F32, tag="xnat", name="xnat")
                    nc.sync.dma_start(xt[:], xd[t * 128 : (t + 1) * 128, :])
                    bstat = sm.tile([128, 2, 6], F32, tag="bstat", name="bstat")
                    for i in range(2):
                        nc.vector.bn_stats(
                            bstat[:, i, :], xt[:, i * 512 : (i + 1) * 512]
                        )
                    nc.vector.bn_aggr(mv[:, t, :], bstat[:])
                rstd = res.tile([128, T], F32, tag=f"rstd_{name}", name=f"rstd_{name}")
                logv = sm.tile([128, T], F32, tag="logv", name="logv")
                nc.vector.tensor_scalar(
                    logv[:], mv[:, :, 1], EPS, None, op0=mybir.AluOpType.add
                )
                nc.scalar.activation(logv[:], logv[:], AF.Ln)
                nc.scalar.activation(rstd[:], logv[:], AF.Exp, scale=-0.5)
                stats[name] = rstd
                pk = sm.tile([128, 2 * T], F32R, tag=f"pk_{name}", name=f"pk_{name}")
                pkv = pk[:].rearrange("p (t two) -> p t two", two=2)
                nc.vector.tensor_scalar_mul(pkv[:, :, 0], mv[:, :, 0], -1.0)
                nc.vector.reciprocal(pkv[:, :, 1], rstd[:])
                rows = res.tile(
                    [2, T, 128], BF16, tag=f"augl_{name}", name=f"augl_{name}"
                )
                for t in range(T):
                    pst = ps1.tile([2, 128], F32R, tag="tp0", name="pst")
                    nc.tensor.transpose(pst[:], pk[:, 2 * t : 2 * t + 2], ident[:])
                    nc.vector.tensor_copy(rows[:, t, :], pst[:])
                stats[name + "_augl"] = rows

            stp_cm.__exit__(None, None, None)

            # ---------- projections (weight prep fused per projection) ----------
            with tc.tile_pool(name="attn", bufs=1) as attnp, tc.tile_pool(
                name="knp", bufs=2
            ) as knp:
              with tc.tile_pool(name="wp", bufs=1) as wpp:
                  QT = attnp.tile([128, NDT, LH], BF16, tag="QT", name="QT")
                  KT = attnp.tile([128, NDT, S], BF16, tag="KT", name="KT")
                  Vaug = attnp.tile(
                      [128, NST, H, E + 1], BF16, tag="Vaug", name="Vaug"
                  )
                  nc.vector.memset(Vaug[:, :, :, E : E + 1], 1.0)

                  for name, Wd, bias_r, xTd, T in (
                      ("q", Wq, bq_r, qT, NLT),
                      ("k", Wk, bk_r, kT, NST),
                      ("v", Wv, bv_r, vT, NST),
                  ):
                      wp = wpp.tile([128, NKT, HE], F32R, tag="Wp", name="Wp")
                      ar = _prep_weight(
                          nc, wk, sm, ps, res, Wd, nwc, nbc, bias_r, wp, name
                      )
                      augl = stats[name + "_augl"]
                      rstd = stats[name]
                      for tg in range(T // 2 if "proj" not in skip else 0):
                          xtw = xs.tile([128, NKT, 256], F32R, tag="xtw", name="xtw")
                          nc.sync.dma_start(
                              xtw[:],
                              xTd.ap().rearrange("(k p) s -> p k s", p=128)[
                                  :, :, tg * 256 : (tg + 1) * 256
                              ],
                          )
                          for ti in range(2):
                              t = tg * 2 + ti
                              po = ps.tile(
                                  [128, HE], F32, tag=f"proj{ti}", name=f"proj{ti}"
                              )
                              for c in range(2):
                                  cs = slice(c * 512, (c + 1) * 512)
                                  for k in range(NKT):
                                      nc.tensor.matmul(
                                          po[:, cs],
                                          xtw[:, k, ti * 128 : (ti + 1) * 128],
                                          wp[:, k, cs],
                                          start=(k == 0),
                                          stop=False,
                                      )
                                  nc.tensor.matmul(
                                      po[:, cs],
                                      augl[:, t, :],
                                      ar[:, cs],
                                      start=False,
                                      stop=True,
                                  )
                              rc = rstd[:, t : t + 1]
                              if name == "q":
                                  nc.scalar.activation(
                                      out1[t][:], po[:], AF.Copy, scale=rc
                                  )
                              elif name == "v":
                                  for c in range(2):
                                      nc.scalar.activation(
                                          Vaug[:, t, 8 * c : 8 * c + 8, 0:E],
                                          po[:, c * 512 : (c + 1) * 512].rearrange(
                                              "p (h e) -> p h e", e=E
                                          ),
                                          AF.Copy,
                                          scale=rc,
                                      )
                              else:
                                  knat = knp.tile(
                                      [128, HE], BF16, tag="knat", name="knat"
                                  )
                                  nc.scalar.activation(
                                      knat[:], po[:], AF.Copy, scale=rc
                                  )
                                  for g in range(2):
                                      pt = ps1.tile(
                                          [128, 512], BF16, tag=f"tp{g}", name="tpb"
                                      )
                                      for j in range(4):
                                          i = g * 4 + j
                                          nc.tensor.transpose(
                                              pt[:, j * 128 : (j + 1) * 128],
                                              knat[:, i * 128 : (i + 1) * 128],
                                              identb[:],
                                          )
                                      nc.vector.tensor_copy(
                                          KT[
                                              :,
                                              g * 4 : (g + 1) * 4,
                                              t * 128 : (t + 1) * 128,
                                          ],
                                          pt[:].rearrange("p (j f) -> p j f", f=128),
                                      )

                  # transpose Q (out1) -> QT
                  for t in range(NLT):
                      for g in range(2):
                          pt = ps1.tile([128, 512], F32R, tag=f"tp{g}", name="tp")
                          for j in range(4):
                              i = g * 4 + j
                              nc.tensor.transpose(
                                  pt[:, j * 128 : (j + 1) * 128],
                                  out1[t][:, i * 128 : (i + 1) * 128],
                                  ident[:],
                              )
                          nc.vector.tensor_copy(
                              QT[:, g * 4 : (g + 1) * 4, t * 128 : (t + 1) * 128],
                              pt[:].rearrange("p (j f) -> p j f", f=128),
                          )

              # ---------- attention ----------
              with tc.tile_pool(name="pt", bufs=2) as ptp:
                  for hi in range(H // 2):  # head pairs, row-group packed
                      for lc in range(2):
                          PTs = []
                          for sh in range(2):
                              PT = ptp.tile(
                                  [128, NST // 2, 1024], BF16, tag="PT", name="PT"
                              )
                              for st8 in range(NST // 2 if "scores" not in skip else 0):
                                  st = sh * 8 + st8
                                  pS = ps.tile(
                                      [128, 1024],
                                      F32,
                                      tag=f"proj{st % 2}",
                                      name="scores",
                                  )
                                  # even head on PE rows 0-63, odd on 64-127:
                                  # consecutive MMs overlap on disjoint row groups
                                  nc.tensor.matmul(
                                      pS[:, 0:512],
                                      KT[0:64, hi, st * 128 : (st + 1) * 128],
                                      QT[0:64, hi, lc * 512 : (lc + 1) * 512],
                                      start=True,
                                      stop=True,
                                  )
                                  nc.tensor.matmul(
                                      pS[:, 512:1024],
                                      KT[64:128, hi, st * 128 : (st + 1) * 128],
                                      QT[64:128, hi, lc * 512 : (lc + 1) * 512],
                                      start=True,
                                      stop=True,
                                  )
                                  nc.scalar.activation(
                                      PT[:, st8, :], pS[:], AF.Exp, scale=SCALE
                                  )
                              PTs.append(PT)
                          for ti in range(4 if "av" not in skip else 0):
                              t = lc * 4 + ti
                              for hh in range(2):
                                  h = 2 * hi + hh
                                  pO = ps1.tile(
                                      [128, E + 1],
                                      F32,
                                      tag=f"av{(2 * ti + hh) % 2}",
                                      name="av",
                                  )
                                  for sh in range(2):
                                      for st8 in range(NST // 2):
                                          nc.tensor.matmul(
                                              pO[:],
                                              PTs[sh][
                                                  :,
                                                  st8,
                                                  hh * 512
                                                  + ti * 128 : hh * 512
                                                  + (ti + 1) * 128,
                                              ],
                                              Vaug[:, sh * 8 + st8, h, :],
                                              start=(sh == 0 and st8 == 0),
                                              stop=(
                                                  sh == 1 and st8 == NST // 2 - 1
                                              ),
                                          )
                                  rc = sm.tile([128, 1], F32, tag="rc", name="rc")
                                  nc.vector.reciprocal(rc[:], pO[:, E : E + 1])
                                  tmp = sm.tile(
                                      [128, E], F32R, tag="avtmp", name="avtmp"
                                  )
                                  nc.vector.tensor_scalar_mul(
                                      tmp[:], pO[:, 0:E], rc[:]
                                  )
                                  nc.vector.tensor_add(
                                      out1[t][:, h * E : (h + 1) * E],
                                      out1[t][:, h * E : (h + 1) * E],
                                      tmp[:],
                                  )


            # spill out1 to DRAM, reload head-mixed:
            # M[h*64+g, j*64+e] = out1[g*16+j, h*64+e]
            for t in range(NLT):
                nc.sync.dma_start(out1d[t * 128 : (t + 1) * 128, :], out1[t][:])
            o1v = out1d.ap().rearrange("(g j) (h e) -> g j h e", j=16, e=64)

            # ---------- final LN + out projection ----------
            with tc.tile_pool(name="fin", bufs=1) as fin:
                M = [
                    fin.tile([128, HE], F32R, tag=f"M_{mt}", name=f"M_{mt}")
                    for mt in range(NLT)
                ]
                for mt in range(NLT):
                    for ho in range(2):
                        nc.sync.dma_start(
                            M[mt][ho * 64 : (ho + 1) * 64, :].rearrange(
                                "g (j e) -> g j e", e=64
                            ),
                            o1v[:, :, 2 * mt + ho, :],
                        )
                n1wc = sm.tile([128, NKT], F32, tag="n1wc", name="n1wc")
                nc.sync.dma_start(
                    n1wc[:], n1w_c.ap().rearrange("(k p) 1 -> p k", p=128)
                )
                n1bc = sm.tile([128, NKT], F32R, tag="n1bc", name="n1bc")
                nc.gpsimd.dma_start(
                    n1bc[:], n1b_c.ap().rearrange("(k p) 1 -> p k", p=128)
                )
                wpo = fin.tile([128, NKT, D], BF16, tag="Wpo", name="Wpo")
                aro = _prep_weight(
                    nc, wk, sm, ps, res, Wo, n1wc, n1bc, bo_r, wpo, "o"
                )

                mv1 = res.tile([128, NLT, 2], F32, tag="mv1", name="mv1")
                for t in range(NLT):
                    bstat = sm.tile([128, 2, 6], F32, tag="bstat", name="bstat")
                    for i in range(2):
                        nc.vector.bn_stats(
                            bstat[:, i, :],
                            M[t][:, i * 512 : (i + 1) * 512].bitcast(F32),
                        )
                    nc.vector.bn_aggr(mv1[:, t, :], bstat[:])
                rstd1 = res.tile([128, NLT], F32, tag="rstd1", name="rstd1")
                logv = sm.tile([128, NLT], F32, tag="logv1", name="logv1")
                nc.vector.tensor_scalar(
                    logv[:], mv1[:, :, 1], EPS, None, op0=mybir.AluOpType.add
                )
                nc.scalar.activation(logv[:], logv[:], AF.Ln)
                nc.scalar.activation(rstd1[:], logv[:], AF.Exp, scale=-0.5)
                pk1 = sm.tile([128, 2 * NLT], F32R, tag="pk1", name="pk1")
                pk1v = pk1[:].rearrange("p (t two) -> p t two", two=2)
                nc.vector.tensor_scalar_mul(pk1v[:, :, 0], mv1[:, :, 0], -1.0)
                nc.vector.tensor_mul(pk1v[:, :, 0], pk1v[:, :, 0], rstd1[:])
                nc.vector.memset(pk1v[:, :, 1].bitcast(F32), 1.0)
                augf = res.tile([2, NLT, 128], BF16, tag="augf", name="augf")
                for t in range(NLT):
                    pst = ps1.tile([2, 128], F32R, tag="tp0", name="pst")
                    nc.tensor.transpose(pst[:], pk1[:, 2 * t : 2 * t + 2], ident[:])
                    nc.vector.tensor_copy(augf[:, t, :], pst[:])

                o1sT = fin.tile([128, NDT, LH], BF16, tag="o1sT", name="o1sT")
                o1T = fin.tile([128, NDT, LH], F32R, tag="o1T", name="o1T")
                for t in range(NLT):
                    o1s = natp.tile([128, HE], F32R, tag="o1s", name="o1s")
                    nc.vector.tensor_scalar_mul(
                        o1s[:], M[t][:], rstd1[:, t : t + 1]
                    )
                    for src, dst in ((o1s, o1sT), (M[t], o1T)):
                        for g in range(2):
                            pt = ps1.tile([128, 512], F32R, tag=f"tp{g}", name="tp")
                            for j in range(4):
                                i = g * 4 + j
                                nc.tensor.transpose(
                                    pt[:, j * 128 : (j + 1) * 128],
                                    src[:, i * 128 : (i + 1) * 128],
                                    ident[:],
                                )
                            nc.vector.tensor_copy(
                                dst[:, g * 4 : (g + 1) * 4, t * 128 : (t + 1) * 128],
                                pt[:].rearrange("p (j f) -> p j f", f=128),
                            )

                for dt in range(NDT if "fin" not in skip else 0):
                    pF = ps.tile([128, LH], F32, tag=f"proj{dt % 2}", name="fin")
                    for lc in range(2):
                        cs = slice(lc * 512, (lc + 1) * 512)
                        for k in range(NKT):
                            nc.tensor.matmul(
                                pF[:, cs],
                                wpo[:, k, dt * 128 : (dt + 1) * 128],
                                o1sT[:, k, cs],
                                start=(k == 0),
                                stop=False,
                            )
                        for t in range(lc * 4, lc * 4 + 4):
                            nc.tensor.matmul(
                                pF[:, t * 128 : (t + 1) * 128],
                                aro[:, dt * 128 : (dt + 1) * 128],
                                augf[:, t, :],
                                start=False,
                                stop=(t == lc * 4 + 3),
                            )
                    fout = natp.tile([128, LH], F32, tag="fout", name="fout")
                    nc.vector.tensor_add(fout[:], pF[:], o1T[:, dt, :].bitcast(F32))
                    nc.sync.dma_start(FT[dt * 128 : (dt + 1) * 128, :], fout[:])

    nc.compile()
    return nc


_NC_CACHE = {}


def kernel(**inputs):
    q = np.ascontiguousarray(inputs["q"], dtype=np.float32)
    k = np.ascontiguousarray(inputs["k"], dtype=np.float32)
    v = np.ascontiguousarray(inputs["v"], dtype=np.float32)
    Wq = np.ascontiguousarray(inputs["Wq"], dtype=np.float32)
    Wk = np.ascontiguousarray(inputs["Wk"], dtype=np.float32)
    Wv = np.ascontiguousarray(inputs["Wv"], dtype=np.float32)
    Wo = np.ascontiguousarray(inputs["Wo"], dtype=np.float32)
    nw = np.asarray(inputs["norm_w"], dtype=np.float32)
    nb = np.asarray(inputs["norm_b"], dtype=np.float32)
    n1w = np.asarray(inputs["norm1_w"], dtype=np.float32)
    n1b = np.asarray(inputs["norm1_b"], dtype=np.float32)
    bq = np.asarray(inputs["bq"], dtype=np.float32)
    bk = np.asarray(inputs["bk"], dtype=np.float32)
    bv = np.asarray(inputs["bv"], dtype=np.float32)
    bo = np.asarray(inputs["bo"], dtype=np.float32)

    if "nc" not in _NC_CACHE:
        _NC_CACHE["nc"] = build_nc()
    nc = _NC_CACHE["nc"]

    shared = {
        "Wq": Wq, "Wk": Wk, "Wv": Wv, "Wo": Wo,
        "nw_c": np.ascontiguousarray(nw.reshape(D, 1)),
        "nb_c": np.ascontiguousarray(nb.reshape(D, 1)),
        "n1w_c": np.ascontiguousarray(n1w.reshape(HE, 1)),
        "n1b_c": np.ascontiguousarray(n1b.reshape(HE, 1)),
        "bq_r": np.ascontiguousarray(bq.reshape(1, HE)),
        "bk_r": np.ascontiguousarray(bk.reshape(1, HE)),
        "bv_r": np.ascontiguousarray(bv.reshape(1, HE)),
        "bo_r": np.ascontiguousarray(bo.reshape(1, D)),
    }
    in_maps = []
    for c in range(8):
        b, half = c // 2, c % 2
        qb = np.ascontiguousarray(q[b, half * LH : (half + 1) * LH, :])
        in_maps.append(
            dict(
                shared,
                qT=np.ascontiguousarray(qb.T),
                kT=np.ascontiguousarray(k[b].T),
                vT=np.ascontiguousarray(v[b].T),
                qn_=qb,
                kn_=np.ascontiguousarray(k[b]),
                vn_=np.ascontiguousarray(v[b]),
            )
        )

    import os
    import time as _time

    trace = os.environ.get("KERNEL_TRACE", "0") == "1"
    try:
        res = run_bass_kernel_spmd(
            nc, in_maps, core_ids=list(range(8)), trace=trace
        )
    except ModuleNotFoundError:
        res = run_bass_kernel_spmd(nc, in_maps, core_ids=list(range(8)))
    if trace:
        if res.exec_time_ns is not None:
            print(f"HW exec time: {res.exec_time_ns} ns")
        else:
            # no NTFF hook in this env: wall-clock the cached-executable rerun
            t0 = _time.perf_counter()
            run_bass_kernel_spmd(nc, in_maps, core_ids=list(range(8)))
            dt = time_ns = int((_time.perf_counter() - t0) * 1e9)
            print(f"HW exec time: {time_ns} ns (wall-clock upper bound)")
    out = np.empty((B, L, D), dtype=np.float32)
    # core (b, half) produced mixed rows m = h*64+g -> global r = h*128 + half*64 + g
    m = np.arange(LH)
    for c in range(8):
        b, half = c // 2, c % 2
        r = (m // 64) * 128 + half * 64 + (m % 64)
        out[b, r, :] = res.results[c]["FT"].T
    return out



# revision 2
# speedup vs baseline: 1.5649x; 1.5649x over previous
"""Fused pre-norm attention layer (B=4, L=2048, D=1024, H=16, E=64) on 8 trn2 cores.

v2: per-head-pair pipelined design.
  - bf16 uploads (natural for bn_stats, transposed for matmul operands)
  - LN folded into projections via aug rows; K projected directly
    transposed (KT[f,s]) with a rank-2 aug; rstd_k applied per-partition
    inside the exp
  - scores fp8e4(QT,KT) -> psum f32 [s,l]; exp split between ACT (Exp
    activation -> fp8) and DVE (fast-exp: int8(x*a+b) bitcast fp8e4m3)
  - AV with fp8 DoubleRow (2 s-tiles per matmul), ones-column denominator
  - out1 (residual Q + attention) bf16; per-head-pair spill + head-mixed
    gather + final LN + Wo projection, pipelined with attention
"""

import numpy as np

import concourse.bass as bass
import concourse.mybir as mybir
import concourse.tile as tile
from concourse import bacc
from concourse.bass_utils import run_bass_kernel_spmd
from concourse.masks import make_identity

F32 = mybir.dt.float32
BF16 = mybir.dt.bfloat16
F8E4 = mybir.dt.float8e4
I8 = mybir.dt.int8
I32 = mybir.dt.int32
AF = mybir.ActivationFunctionType
ALU = mybir.AluOpType
DR = mybir.MatmulPerfMode.DoubleRow

B, L, D, H, E = 4, 2048, 1024, 16, 64
LH = L // 2
S = L
HE = H * E
EPS = 1e-5
NKT = D // 128       # 8
NLT = LH // 128      # 8
NST = S // 128       # 16
HP = H // 2          # 8 head pairs
SCALE = 1.0 / np.sqrt(E)
LOG2E = 1.4426950408889634
CEXP = 0.45
DVE_STS = frozenset((3, 5, 7, 11, 13, 15))


def _rsqrt_dve(nc, sm, out, var_ap, T, tag):
    """out = 1/sqrt(var + EPS) entirely on DVE (Quake initial guess + 2
    Newton steps) so the ACT engine never leaves the exp table set."""
    ALUo = mybir.AluOpType
    veps = sm.tile([128, T], F32, tag=f"ve_{tag}", name=f"ve_{tag}")
    nc.vector.tensor_scalar(veps[:], var_ap, EPS, None, op0=ALUo.add)
    s1 = sm.tile([128, T], I32, tag=f"s1_{tag}", name=f"s1_{tag}")
    nc.vector.tensor_scalar(
        s1[:], veps[:].bitcast(I32), 1, None, op0=ALUo.logical_shift_right
    )
    nc.vector.tensor_scalar(
        s1[:], s1[:], -1, 0x5F3759DF, op0=ALUo.mult, op1=ALUo.add
    )
    r = s1[:].bitcast(F32)
    t = sm.tile([128, T], F32, tag=f"t_{tag}", name=f"t_{tag}")
    for _ in range(2):
        nc.vector.tensor_mul(t[:], veps[:], r)
        nc.vector.tensor_mul(t[:], t[:], r)
        nc.vector.tensor_scalar(t[:], t[:], -0.5, 1.5, op0=ALUo.mult, op1=ALUo.add)
        nc.vector.tensor_mul(out, r, t[:])
        r = out
    return out


def build_nc(skip=()):
    nc = bacc.Bacc("TRN2", target_bir_lowering=False, debug=False, num_devices=8)

    qn_ = nc.dram_tensor("qn_", [LH, D], BF16, kind="ExternalInput")
    kn_ = nc.dram_tensor("kn_", [S, D], BF16, kind="ExternalInput")
    vn_ = nc.dram_tensor("vn_", [S, D], BF16, kind="ExternalInput")
    qT = nc.dram_tensor("qT", [D, LH], BF16, kind="ExternalInput")
    kT = nc.dram_tensor("kT", [D, S], F8E4, kind="ExternalInput")
    vT = nc.dram_tensor("vT", [D, S], F8E4, kind="ExternalInput")
    Wq = nc.dram_tensor("Wq", [D, HE], BF16, kind="ExternalInput")
    Wk = nc.dram_tensor("Wk", [D, HE], F8E4, kind="ExternalInput")
    Wv = nc.dram_tensor("Wv", [D, HE], F8E4, kind="ExternalInput")
    Wo = nc.dram_tensor("Wo", [HE, D], BF16, kind="ExternalInput")
    nw_c = nc.dram_tensor("nw_c", [D, 1], F32, kind="ExternalInput")
    nb_c = nc.dram_tensor("nb_c", [D, 1], BF16, kind="ExternalInput")
    n1w_c = nc.dram_tensor("n1w_c", [HE, 1], F32, kind="ExternalInput")
    n1b_c = nc.dram_tensor("n1b_c", [HE, 1], BF16, kind="ExternalInput")
    bq_r = nc.dram_tensor("bq_r", [1, HE], F32, kind="ExternalInput")
    bk_r = nc.dram_tensor("bk_r", [1, HE], F32, kind="ExternalInput")
    bv_r = nc.dram_tensor("bv_r", [1, HE], F32, kind="ExternalInput")
    bo_r = nc.dram_tensor("bo_r", [1, D], F32, kind="ExternalInput")
    out1d = nc.dram_tensor("out1d", [LH, HE], BF16)
    F = nc.dram_tensor("F", [LH, D], F32, kind="ExternalOutput")

    with tile.TileContext(nc) as tc, nc.allow_low_precision(
        reason="bf16/fp8 attention; tolerance budget verified vs reference"
    ):
        with (
            tc.tile_pool(name="res", bufs=1) as res,
            tc.tile_pool(name="sm", bufs=2) as sm,
        ):
            identb = res.tile([128, 128], BF16, tag="identb", name="identb")
            make_identity(nc, identb[:])
            ones_row = res.tile([1, 128], BF16, tag="ones_row", name="ones_row")
            nc.vector.memset(ones_row[:], 1.0)

            nwc = res.tile([128, NKT], F32, tag="nwc", name="nwc")
            nc.sync.dma_start(nwc[:], nw_c.ap().rearrange("(k p) 1 -> p k", p=128))
            nbc = res.tile([128, NKT], BF16, tag="nbc", name="nbc")
            nc.sync.dma_start(nbc[:], nb_c.ap().rearrange("(k p) 1 -> p k", p=128))
            n1wc = res.tile([128, NKT], F32, tag="n1wc", name="n1wc")
            nc.sync.dma_start(n1wc[:], n1w_c.ap().rearrange("(k p) 1 -> p k", p=128))
            n1bc = res.tile([128, NKT], BF16, tag="n1bc", name="n1bc")
            nc.sync.dma_start(n1bc[:], n1b_c.ap().rearrange("(k p) 1 -> p k", p=128))
            onesb = res.tile([128, 1], BF16, tag="onesb", name="onesb")
            nc.vector.memset(onesb[:], 1.0)
            onesb8 = res.tile([128, 1], F8E4, tag="onesb8", name="onesb8")
            nc.vector.memset(onesb8[:], 1.0)
            nbc8 = res.tile([128, NKT], F8E4, tag="nbc8", name="nbc8")

            kTt = res.tile([128, NKT, S], F8E4, tag="kTt", name="kTt")

            out1 = res.tile([128, NLT, HE], BF16, tag="out1", name="out1")
            QT = res.tile([128, HP, LH], F8E4, tag="QT", name="QT")
            Vaug = res.tile([128, NST, H, E + 1], F8E4, tag="Vaug", name="Vaug")
            nc.vector.memset(Vaug[:, :, :, E : E + 1], 1.0)
            wpk = res.tile([128, NKT, HE], F8E4, tag="wpk", name="wpk")
            wpo = res.tile([128, NKT, D], BF16, tag="wpo", name="wpo")
            aug = {}   # name -> rows tile [2, T, 128] (free-layout [-mu; 1/rstd])
            ar = {}    # name -> [2, n] rows [S=colsum(wp); c=nb@W+bias]
            ar_c = {}  # name -> [1, n] c-row at base partition 0
            rstd = {}  # name -> [128, T]

            with tc.tile_pool(name="wqvw", bufs=1) as wqvw:
              with (
                  tc.tile_pool(name="pp", bufs=2, space="PSUM") as pp,
                  tc.tile_pool(name="wk", bufs=1) as wk,
              ):
                # ---------- LN stats ----------
                def do_stats(name, xd, T, psum_pool, psum_tag, stp):
                    mv = res.tile(
                        [128, T, 2], F32, tag=f"mv_{name}", name=f"mv_{name}"
                    )
                    for t in range(T):
                        xt = stp.tile(
                            [128, D], BF16, tag=f"xnat{t % 6}", name="xnat"
                        )
                        nc.scalar.dma_start(xt[:], xd[t * 128 : (t + 1) * 128, :])
                        bstat = sm.tile([128, 2, 6], F32, tag="bstat", name="bstat")
                        for i in range(2):
                            nc.vector.bn_stats(
                                bstat[:, i, :], xt[:, i * 512 : (i + 1) * 512]
                            )
                        nc.vector.bn_aggr(mv[:, t, :], bstat[:])
                    rs = res.tile(
                        [128, T], F32, tag=f"rstd_{name}", name=f"rstd_{name}"
                    )
                    _rsqrt_dve(nc, sm, rs[:], mv[:, :, 1], T, "st")
                    rstd[name] = rs
                    pk = sm.tile(
                        [128, 2 * T], BF16, tag=f"pk_{name}", name=f"pk_{name}"
                    )
                    pkv = pk[:].rearrange("p (t two) -> p t two", two=2)
                    nc.vector.tensor_scalar_mul(pkv[:, :, 0], mv[:, :, 0], -1.0)
                    rcp = sm.tile([128, T], F32, tag="rcp", name="rcp")
                    nc.vector.reciprocal(rcp[:], rs[:])
                    nc.vector.tensor_copy(pkv[:, :, 1], rcp[:])
                    rows = res.tile(
                        [2, T, 128], BF16, tag=f"augl_{name}", name=f"augl_{name}"
                    )
                    for t in range(T):
                        pst = psum_pool.tile([2, 128], BF16, tag=psum_tag, name="pst")
                        nc.tensor.transpose(
                            pst[:], pk[:, 2 * t : 2 * t + 2], identb[:]
                        )
                        nc.vector.tensor_copy(rows[:, t, :], pst[:])
                    aug[name] = rows

                do_stats("q", qn_, NLT, pp, "tp", wk)

                # ---------- weight prep (fold LN weight, build ar rows) ----------
                def prep_w(Wd, bias_r, wp, wvec, bvec, tag):
                    n = Wd.shape[1]
                    dt = Wd.dtype
                    ob = onesb8 if dt == F8E4 else onesb
                    wt = wk.tile([128, NKT, n], dt, tag="wtile", name=f"wt_{tag}")
                    nc.sync.dma_start(
                        wt[:], Wd.ap().rearrange("(k p) n -> p k n", p=128)
                    )
                    ps_c = pp.tile([1, 1024], F32, tag="prow", name=f"pc_{tag}")
                    ps_s = pp.tile([1, 1024], F32, tag="prow", name=f"ps_{tag}")
                    for k in range(NKT):
                        eng = nc.gpsimd if k % 2 == 0 else nc.vector
                        eng.tensor_scalar_mul(
                            wp[:, k, :], wt[:, k, :], wvec[:, k : k + 1]
                        )
                        for c in range(2):
                            cs = slice(c * 512, (c + 1) * 512)
                            nc.tensor.matmul(
                                ps_c[:, cs], bvec[:, k : k + 1], wt[:, k, cs],
                                start=(k == 0), stop=(k == NKT - 1),
                            )
                            nc.tensor.matmul(
                                ps_s[:, cs], ob, wp[:, k, cs],
                                start=(k == 0), stop=(k == NKT - 1),
                            )
                    a = res.tile([2, n], BF16, tag=f"ar_{tag}", name=f"ar_{tag}")
                    nc.vector.tensor_copy(a[0:1, :], ps_s[:, 0:n])
                    brow = sm.tile([1, n], F32, tag="brow", name=f"brow_{tag}")
                    nc.sync.dma_start(brow[:], bias_r.ap())
                    crow = res.tile([1, n], BF16, tag=f"crow_{tag}", name=f"crow_{tag}")
                    nc.vector.tensor_add(crow[:], ps_c[:, 0:n], brow[:])
                    nc.sync.dma_start(a[1:2, :], crow[:])
                    ar_c[tag] = crow
                    return a

                wpq = wqvw.tile([128, NKT, HE], BF16, tag="wpq", name="wpq")
                ar["q"] = prep_w(Wq, bq_r, wpq, nwc, nbc, "q")
                nc.vector.tensor_copy(nbc8[:], nbc[:])
                ar["k"] = prep_w(Wk, bk_r, wpk, nwc, nbc8, "k")
                wpv = wqvw.tile([128, NKT, HE], F8E4, tag="wpv", name="wpv")
                ar["v"] = prep_w(Wv, bv_r, wpv, nwc, nbc8, "v")
                ar["o"] = prep_w(Wo, bo_r, wpo, n1wc, n1bc, "o")
                nc.sync.dma_start(
                    kTt[:], kT.ap().rearrange("(k p) s -> p k s", p=128)
                )

              # ---------- Q-all projection ----------
              with (
                    tc.tile_pool(name="wq1", bufs=1) as wq1,
                    tc.tile_pool(name="qv", bufs=2, space="PSUM") as qvp,
              ):
                    qTt = wq1.tile([128, NKT, LH], BF16, tag="qTt", name="qTt")
                    nc.scalar.dma_start(
                        qTt[:], qT.ap().rearrange("(k p) s -> p k s", p=128)
                    )
                    for t in range(NLT if "q" not in skip else 0):
                        po = qvp.tile([128, HE], F32, tag="proj", name="poq")
                        for c in range(2):
                            cs = slice(c * 512, (c + 1) * 512)
                            for k in range(NKT):
                                nc.tensor.matmul(
                                    po[:, cs], qTt[:, k, t * 128 : (t + 1) * 128],
                                    wpq[:, k, cs], start=(k == 0), stop=False,
                                )
                            nc.tensor.matmul(
                                po[:, cs], aug["q"][:, t, :], ar["q"][:, cs],
                                start=False, stop=True,
                            )
                        nc.scalar.activation(
                            out1[:, t, :], po[:], AF.Copy,
                            scale=rstd["q"][:, t : t + 1],
                        )
                        for g in range(2):
                            ptq = qvp.tile([128, 512], BF16, tag="tpq", name="ptq")
                            for j in range(4):
                                i = g * 4 + j
                                nc.tensor.transpose(
                                    ptq[:, j * 128 : (j + 1) * 128],
                                    out1[:, t, i * 128 : (i + 1) * 128],
                                    identb[:],
                                )
                            nc.vector.tensor_copy(
                                QT[:, g * 4 : (g + 1) * 4, t * 128 : (t + 1) * 128],
                                ptq[:].rearrange("p (j f) -> p j f", f=128),
                            )
                    # k/v stats stream on DVE/ACT while Q-all runs on PE
                    do_stats("k", kn_, NST, qvp, "tp", wq1)
                    do_stats("v", vn_, NST, qvp, "tp", wq1)
                    sck = res.tile([128, NST], F32, tag="sck", name="sck")
                    nc.vector.tensor_scalar_mul(sck[:], rstd["k"][:], SCALE)
                    akt = res.tile([128, NST], F32, tag="akt", name="akt")
                    nc.vector.tensor_scalar_mul(
                        akt[:], rstd["k"][:], SCALE * LOG2E * 8.0
                    )

              # ---------- V-all projection ----------
              with (
                    tc.tile_pool(name="wv1", bufs=1) as wv1,
                    tc.tile_pool(name="qv2", bufs=2, space="PSUM") as qvp,
              ):
                    vTt = wv1.tile([128, NKT, S], F8E4, tag="vTt", name="vTt")
                    nc.scalar.dma_start(
                        vTt[:], vT.ap().rearrange("(k p) s -> p k s", p=128)
                    )
                    vdr = vTt[:].rearrange("p (kp kt) s -> p kp kt s", kt=2)
                    wvdr = wpv[:].rearrange("p (kp kt) n -> p kp kt n", kt=2)
                    for t in range(NST if "v" not in skip else 0):
                        po = qvp.tile([128, HE], F32, tag="proj", name="pov")
                        for c in range(2):
                            cs = slice(c * 512, (c + 1) * 512)
                            for kp in range(NKT // 2):
                                nc.tensor.matmul(
                                    po[:, cs],
                                    vdr[:, kp, :, t * 128 : (t + 1) * 128],
                                    wvdr[:, kp, :, cs],
                                    start=(kp == 0), stop=False, perf_mode=DR,
                                )
                            nc.tensor.matmul(
                                po[:, cs], aug["v"][:, t, :], ar["v"][:, cs],
                                start=False, stop=True,
                            )
                        nc.scalar.activation(
                            Vaug[:, t, :, 0:E],
                            po[:].rearrange("p (h e) -> p h e", e=E),
                            AF.Copy,
                            scale=rstd["v"][:, t : t + 1],
                        )

            # ---------- attention + final stage, per head pair ----------
            with (
                tc.tile_pool(name="kt", bufs=2) as ktp,
                tc.tile_pool(name="pt", bufs=3) as ptpool,
                tc.tile_pool(name="mst", bufs=2) as mstp,
                tc.tile_pool(name="sc", bufs=2, space="PSUM") as scp,
                tc.tile_pool(name="av", bufs=2, space="PSUM") as avp,
                tc.tile_pool(name="fin", bufs=2, space="PSUM") as finp,
            ):
                fn_state = {}

                def fn_nonpe(mt):
                    """Spill out1 head-pair columns, gather mixed M, LN stats,
                    normalized o1s. DMA/DVE/ACT only — no PE."""
                    nc.sync.dma_start(
                        out1d.ap().rearrange("(t p) f -> p t f", p=128)[
                            :, :, mt * 128 : (mt + 1) * 128
                        ],
                        out1[:, :, mt * 128 : (mt + 1) * 128],
                    )
                    M = mstp.tile([128, HE], BF16, tag="M", name="M")
                    o1v = out1d.ap().rearrange("(g j) (h e) -> g j h e", j=16, e=64)
                    for ho in range(2):
                        nc.sync.dma_start(
                            M[ho * 64 : (ho + 1) * 64, :].rearrange(
                                "g (j e) -> g j e", e=64
                            ),
                            o1v[:, :, 2 * mt + ho, :],
                        )
                    bstat = sm.tile([128, 2, 6], F32, tag="bstatm", name="bstatm")
                    for i in range(2):
                        nc.vector.bn_stats(
                            bstat[:, i, :], M[:, i * 512 : (i + 1) * 512]
                        )
                    mvf = sm.tile([128, 2], F32, tag="mvf", name="mvf")
                    nc.vector.bn_aggr(mvf[:], bstat[:])
                    rstdf = sm.tile([128, 1], F32, tag="rstdf", name="rstdf")
                    _rsqrt_dve(nc, sm, rstdf[:], mvf[:, 1:2], 1, "fn")
                    negmu = sm.tile([128, 1], F32, tag="negmu", name="negmu")
                    nc.vector.tensor_scalar_mul(negmu[:], mvf[:, 0:1], -1.0)
                    o1s = mstp.tile([128, HE], BF16, tag="o1s", name="o1s")
                    nc.vector.tensor_scalar(
                        o1s[:], M[:], negmu[:], rstdf[:], op0=ALU.add, op1=ALU.mult
                    )
                    fn_state[mt] = (M, o1s)

                def fn_pe_thunks(mt):
                    """Final-stage PE work for row-tile mt as 4 independent
                    chunks to interleave between attention score groups."""
                    M, o1s = fn_state.pop(mt)
                    o1sT = mstp.tile([128, NKT, 128], BF16, tag="o1sT", name="o1sT")

                    def tp(g):
                        def run():
                            pst = finp.tile([128, 512], BF16, tag="f", name="tpf")
                            for j in range(4):
                                i = g * 4 + j
                                nc.tensor.transpose(
                                    pst[:, j * 128 : (j + 1) * 128],
                                    o1s[:, i * 128 : (i + 1) * 128],
                                    identb[:],
                                )
                            nc.vector.tensor_copy(
                                o1sT[:, g * 4 : (g + 1) * 4, :],
                                pst[:].rearrange("p (j f) -> p j f", f=128),
                            )
                        return run

                    def db_run(db):
                        def run():
                            ds = slice(db * 512, (db + 1) * 512)
                            fnp = finp.tile([128, 512], F32, tag="f", name="fnp")
                            for k in range(NKT):
                                nc.tensor.matmul(
                                    fnp[:], o1sT[:, k, :], wpo[:, k, ds],
                                    start=(k == 0), stop=False,
                                )
                            nc.tensor.matmul(
                                fnp[:], ones_row[:], ar_c["o"][:, ds],
                                start=False, stop=True,
                            )
                            fout = mstp.tile([128, 512], F32, tag="fout", name="fout")
                            nc.vector.tensor_add(fout[:], fnp[:], M[:, ds])
                            nc.sync.dma_start(
                                F[mt * 128 : (mt + 1) * 128, ds], fout[:]
                            )
                        return run

                    return [tp(0), tp(1), db_run(0), db_run(1)]

                def k_thunks(hp, KTh):
                    """K projection for head pair hp as 4 chunks."""
                    fsl = slice(hp * 128, (hp + 1) * 128)

                    kdr = kTt[:].rearrange("p (kp kt) s -> p kp kt s", kt=2)
                    wkdr = wpk[:].rearrange("p (kp kt) n -> p kp kt n", kt=2)

                    def chain(sq):
                        def run():
                            ss = slice(sq * 512, (sq + 1) * 512)
                            pkps = finp.tile([128, 512], F32, tag="f", name="kproj")
                            for kp in range(NKT // 2):
                                nc.tensor.matmul(
                                    pkps[:], wkdr[:, kp, :, fsl],
                                    kdr[:, kp, :, ss],
                                    start=(kp == 0), stop=False, perf_mode=DR,
                                )
                            nc.tensor.matmul(
                                pkps[:], ar["k"][:, fsl],
                                aug["k"][:, sq * 4 : (sq + 1) * 4, :],
                                start=False, stop=True,
                            )
                            nc.scalar.activation(KTh[:, ss], pkps[:], AF.Copy)
                        return run

                    return [chain(sq) for sq in range(4)]

                def attend(hp, h01, KTh, pe_fillers):
                    h = 2 * hp + h01
                    hrow = slice(h01 * 64, (h01 + 1) * 64)
                    pava = avp.tile([128, 4, E + 1], F32, tag="av", name="pava")
                    pavb = avp.tile([128, 4, E + 1], F32, tag="av", name="pavb")
                    pts = {}

                    def av_chain(stp):
                        PT = pts.pop(stp)
                        for lsub in range(NLT):
                            pav = pava if lsub < 4 else pavb
                            nc.tensor.matmul(
                                pav[:, lsub % 4, :],
                                PT[:, :, lsub * 128 : (lsub + 1) * 128],
                                Vaug[:, 2 * stp : 2 * stp + 2, h, :],
                                start=(stp == 0), stop=(stp == NST // 2 - 1),
                                perf_mode=DR,
                            )

                    for stp in range(NST // 2):
                        PT = ptpool.tile([128, 2, LH], F8E4, tag="PT", name="PT")
                        pts[stp] = PT
                        for j in range(2):
                            st = 2 * stp + j
                            psc = scp.tile([128, LH], F32, tag="sc", name="psc")
                            for lc in range(2):
                                ls = slice(lc * 512, (lc + 1) * 512)
                                nc.tensor.matmul(
                                    psc[:, ls],
                                    KTh[hrow, st * 128 : (st + 1) * 128],
                                    QT[hrow, hp, ls],
                                    start=True, stop=True,
                                )
                            if st in DVE_STS:
                                nc.vector.tensor_scalar(
                                    PT[:, j, :].bitcast(I8), psc[:],
                                    akt[:, st : st + 1], 56.0 - CEXP,
                                    op0=ALU.mult, op1=ALU.add,
                                )
                            else:
                                nc.scalar.activation(
                                    PT[:, j, :], psc[:], AF.Exp,
                                    scale=sck[:, st : st + 1],
                                )
                        if pe_fillers:
                            pe_fillers.pop(0)()
                        if stp >= 1:
                            av_chain(stp - 1)
                    av_chain(NST // 2 - 1)
                    for half, pav in ((0, pava), (1, pavb)):
                        rc4 = sm.tile([128, 4], F32, tag="rc4", name="rc4")
                        nc.vector.reciprocal(rc4[:], pav[:, :, E])
                        for i in range(4):
                            lsub = half * 4 + i
                            nc.vector.scalar_tensor_tensor(
                                out1[:, lsub, h * E : (h + 1) * E],
                                pav[:, i, 0:E],
                                rc4[:, i : i + 1],
                                out1[:, lsub, h * E : (h + 1) * E],
                                op0=ALU.mult, op1=ALU.add,
                            )

                # K(0) projected up front; K(hp+1) and FN(hp-1) PE work are
                # interleaved into attention score slots as fillers.
                KThs = {0: ktp.tile([128, S], F8E4, tag="KT", name="KT0")}
                for t in k_thunks(0, KThs[0]):
                    t()
                for hp in range(HP):
                    fill_h0, fill_h1 = [], []
                    if hp + 1 < HP:
                        KThs[hp + 1] = ktp.tile([128, S], F8E4, tag="KT", name="KT")
                        fill_h0 = k_thunks(hp + 1, KThs[hp + 1])
                    if hp > 0 and "fin" not in skip:
                        fill_h1 = fn_pe_thunks(hp - 1)
                    if "attn" not in skip:
                        attend(hp, 0, KThs[hp], fill_h0)
                        attend(hp, 1, KThs[hp], fill_h1)
                    else:
                        for t in fill_h0 + fill_h1:
                            t()
                    del KThs[hp]
                    if "fin" not in skip:
                        fn_nonpe(hp)
                if "fin" not in skip:
                    for t in fn_pe_thunks(HP - 1):
                        t()

    nc.compile()
    return nc


_NC_CACHE = {}


def kernel(**inputs):
    import ml_dtypes

    bf16 = ml_dtypes.bfloat16
    q = np.asarray(inputs["q"], dtype=np.float32)
    k = np.asarray(inputs["k"], dtype=np.float32)
    v = np.asarray(inputs["v"], dtype=np.float32)

    if "nc" not in _NC_CACHE:
        _NC_CACHE["nc"] = build_nc()
    nc = _NC_CACHE["nc"]

    f8 = mybir.dt.np(F8E4)
    shared = {
        "Wq": np.asarray(inputs["Wq"], dtype=bf16),
        "Wk": np.asarray(inputs["Wk"], dtype=np.float32).astype(f8),
        "Wv": np.asarray(inputs["Wv"], dtype=np.float32).astype(f8),
        "Wo": np.asarray(inputs["Wo"], dtype=bf16),
        "nw_c": np.ascontiguousarray(
            np.asarray(inputs["norm_w"], np.float32).reshape(D, 1)
        ),
        "nb_c": np.asarray(inputs["norm_b"], np.float32).astype(bf16).reshape(D, 1),
        "n1w_c": np.ascontiguousarray(
            np.asarray(inputs["norm1_w"], np.float32).reshape(HE, 1)
        ),
        "n1b_c": np.asarray(inputs["norm1_b"], np.float32).astype(bf16).reshape(HE, 1),
        "bq_r": np.asarray(inputs["bq"], np.float32).reshape(1, HE),
        "bk_r": np.asarray(inputs["bk"], np.float32).reshape(1, HE),
        "bv_r": np.asarray(inputs["bv"], np.float32).reshape(1, HE),
        "bo_r": np.asarray(inputs["bo"], np.float32).reshape(1, D),
    }
    in_maps = []
    for c in range(8):
        b, half = c // 2, c % 2
        qb = np.ascontiguousarray(q[b, half * LH : (half + 1) * LH, :])
        in_maps.append(
            dict(
                shared,
                qn_=qb.astype(bf16),
                kn_=k[b].astype(bf16),
                vn_=v[b].astype(bf16),
                qT=np.ascontiguousarray(qb.T).astype(bf16),
                kT=np.ascontiguousarray(k[b].T).astype(f8),
                vT=np.ascontiguousarray(v[b].T).astype(f8),
            )
        )

    res = run_bass_kernel_spmd(nc, in_maps, core_ids=list(range(8)))
    out = np.empty((B, L, D), dtype=np.float32)
    m = np.arange(LH)
    r = (m // 64) * 128 + (m % 64)
    for c in range(8):
        b, half = c // 2, c % 2
        out[b, r + half * 64, :] = res.results[c]["F"]
    return out
